# revision 4
# baseline (speedup 1.0000x reference)
"""Trainium2 Bass kernel for nn_Node2Property2 (segment_reduce), v2.

Model: out = segment_sum(softplus_shifted(x @ W1 + b1) @ W2, batch, G).

v2 strategy (8 cores, data-parallel over nodes; ~2x the v1 throughput):
  - x is shipped as fp8 e3m4 (2x pre-scale), W1 stationary as e3m4 (16x):
    halves input DMA vs bf16; mm1 runs fp8 with FWL.
  - Per 1024-node slot, v=W1q.T@xq lands in a [128,1024] f32 PSUM tile.
    "A" slots: ScalarE silu(C*v/32 + cb) -> bf16, the silu part of a
    softplus fit (AL*silu(C a+D) + GM*a + BE ~ softplus(a)).
    "P" slots: ONE custom DVE instruction evaluates an even cubic-in-t
    (t = min(v^2, T2)) fit of psi(a) = softplus(a) - a/2 straight from
    PSUM -> bf16. This offloads ~3/8 of the activation work from the
    (otherwise saturated) ScalarE to the Vector engine.
  - Both paths write h in a per-slot split layout [evens 512 | odds 512];
    a bf16 2x tensor_add forms pair sums ph (adjacent nodes share a graph
    except at segment boundaries -> fixed exactly on host).
  - mm2: per slot one [H,8]-stationary matmul (w2 in column s) streams
    ph (512 cols, HALF of v1's per-node count) accumulating into an
    [8,512] stripe of one persistent [128,512] PSUM tile (partition
    offset 8*group); a single DVE copy + DMA evicts all 65536 pair sums.
  - Host: segment-sum of pair sums; pairs straddling a segment boundary
    are recomputed exactly in f64; linear/constant folds (incl. the
    fp8-quantisation mean correction and the mm2-weight rounding
    correction) close the gap to softplus. Simulated rel err ~7.6e-3.

kernel(**inputs) takes FULL inputs, returns the FULL [G, 1] f32 output.
"""

import os
import sys

for _p in ("/opt/trn_rl_repo", "/root/.axon_site/_ro/trn_rl_repo"):
    if os.path.isdir(_p) and _p not in sys.path:
        sys.path.insert(0, _p)

import numpy as np
import ml_dtypes

import concourse.bacc as bacc
import concourse.mybir as mybir
import concourse.tile as tile
from concourse.bass_utils import run_bass_kernel_spmd

F32 = mybir.dt.float32
BF16 = mybir.dt.bfloat16
FP8E3 = mybir.dt.float8e3
AF = mybir.ActivationFunctionType

E3 = ml_dtypes.float8_e3m4
BF = ml_dtypes.bfloat16

LOG2 = float(np.log(2.0))

# softplus(a) ~= AL*silu(C_*a + D_) + GM*a + BE  (v1's L2 fit, A-path)
AL = 1.16340907
C_ = 0.65158221
D_ = 6.08993352e-04
GM = 0.12077211
BE = 0.69315987

# Problem shape.
N, IN, H, OUT, G = 1048576, 128, 128, 1, 16384
NCORES = 8
NC_NODES = N // NCORES          # 131072 nodes per core

# Device tiling.
NS = 1024                       # nodes per slot
SLOTS = 8                       # slots per group
GRP = NS * SLOTS                # 8192 nodes per group
NGRP = NC_NODES // GRP          # 16 groups per core
PATTERN = "AAPAAPAP"            # 5 silu slots, 3 poly slots per group
XT_SPLIT = 4                    # input-DMA split per group
SC = 32.0                       # x*2 and W1*16 pre-scales (v_dev = 32 v)

# ---- P-path cubic fit: psi(u)=softplus(u)-u/2 ~ c0+c1 t+c2 t^2+c3 t^3,
#      t = min(u^2, T^2), Gaussian-weighted on [0, T] ----
_T = 6.5


def _fit_poly():
    u = np.linspace(0.0, _T, 40001)
    t = u * u
    psi = np.logaddexp(0.0, u) - 0.5 * u
    w = np.exp(-0.5 * u * u) + 3e-5
    A = np.stack([t**k for k in range(4)], axis=1)
    coef, *_ = np.linalg.lstsq(A * np.sqrt(w)[:, None], psi * np.sqrt(w),
                               rcond=None)
    return coef


_COEF = _fit_poly()
C0P = float(_COEF[0])
# device constants act on v_dev = SC*v (t_dev = v_dev^2), output /AL since
# the mm2 weights carry AL*W2
C1D = float(_COEF[1] / AL / SC**2)
C2D = float(_COEF[2] / AL / SC**4)
C3D = float(_COEF[3] / AL / SC**6)
T2D = float(_T * _T * SC * SC)

# ---- custom DVE op registration ----
from concourse.dve_spec import (Spec, Src0, C0, C1, C2, C3, minn, sq, lower,
                                _spill_c3_to_src1)
from concourse.dve_ops import (DveOp, OPS, CUSTOM_DVE_SPECS,
                               _SUB_OPCODE_FOR_NAME, _CUSTOM_DVE_ROW_BASE)
from concourse.dve_uop import DveOpSpec


def _register_poly_op():
    """out = ((s0*t + s1)*t + imm2)*t, t = min(sq(in0), C3 via in1)."""
    t = minn(sq(Src0), C3)
    body = _spill_c3_to_src1(((C0 * t + C1) * t + C2) * t)

    def ref(in0, in1, s0, s1, imm2):
        tt = np.minimum(np.float32(in0) * np.float32(in0), in1)
        return ((s0 * tt + s1) * tt + imm2) * tt

    op = DveOp.__new__(DveOp)
    object.__setattr__(op, "name", "SOFTPLUS_EVEN_P3")
    object.__setattr__(op, "spec", Spec(body=body, reference=ref))
    object.__setattr__(op, "subdim", False)
    object.__setattr__(op, "perf_en", {})
    shas = {}
    for ver in ("v3", "v4"):
        tmp = DveOpSpec(name=op.name, opcode=0, uops=lower(op.spec, ver=ver),
                        rd1_en=True)
        shas[ver] = tmp.sha(ver)
    object.__setattr__(op, "uops_sha", shas)
    if op.name not in _SUB_OPCODE_FOR_NAME:
        OPS.append(op)
        CUSTOM_DVE_SPECS[op.name] = op.spec
        _SUB_OPCODE_FOR_NAME[op.name] = _CUSTOM_DVE_ROW_BASE + len(OPS) - 1
    return op


_POLY = _register_poly_op()


def _build_nc(repeat=1):
    nc = bacc.Bacc("TRN2", target_bir_lowering=False, debug=False,
                   num_devices=NCORES)
    xT = nc.declare_dram_parameter("xT", [IN, NC_NODES], FP8E3, isOutput=False)
    W1 = nc.declare_dram_parameter("W1", [IN, H], FP8E3, isOutput=False)
    W2B = nc.declare_dram_parameter("W2B", [H, 64], BF16, isOutput=False)
    CB = nc.declare_dram_parameter("CB", [H, 1], F32, isOutput=False)
    T2T = nc.declare_dram_parameter("T2T", [H, 1], F32, isOutput=False)
    s_out = nc.declare_dram_parameter("s", [128, 512], F32, isOutput=True)

    with tile.TileContext(nc) as tc:
        with (
            tc.tile_pool(name="wts", bufs=1) as wts,
            tc.tile_pool(name="xp", bufs=3) as xp,
            tc.tile_pool(name="hp", bufs=2) as hp,
            tc.tile_pool(name="php", bufs=2) as php,
            tc.tile_pool(name="stp", bufs=2) as stp,
            tc.tile_pool(name="vps", bufs=3, space="PSUM") as vps,
            tc.tile_pool(name="sps", bufs=2, space="PSUM") as sps,
        ):
            w1r = wts.tile([IN, H], FP8E3)
            w2r = wts.tile([H, 64], BF16)
            cbt = wts.tile([H, 1], F32)
            t2s = wts.tile([H, 1], F32)
            nc.sync.dma_start(w1r[:], W1[:])
            nc.sync.dma_start(w2r[:], W2B[:])
            nc.sync.dma_start(cbt[:], CB[:])
            nc.sync.dma_start(t2s[:], T2T[:])
            # stage weights via DVE so matmuls wait on one producer
            w1t = wts.tile([IN, H], FP8E3)
            nc.vector.tensor_copy(w1t[:], w1r[:])
            w2t = wts.tile([H, 64], BF16)
            nc.vector.tensor_copy(w2t[:], w2r[:])
            # warm the silu table during the DMA ramp
            warm = wts.tile([H, 1], F32)
            nc.scalar.activation(warm[:], cbt[:], AF.Silu, bias=0.0, scale=1.0)

            state = {"spt": None}
            pending = []        # (g, s, ph_tile) mm2 not yet emitted

            def emit_mm2(n):
                for _ in range(n):
                    if not pending:
                        return
                    g, s, pht = pending.pop(0)
                    if s == 0:
                        state["spt"] = sps.tile([8, 512], F32, name="spt")
                    spt = state["spt"]
                    nc.tensor.matmul(
                        spt[:], w2t[:, 8 * s:8 * s + 8],
                        pht[:, 512 * s:512 * (s + 1)],
                        start=(s == 0), stop=(s == SLOTS - 1))
                    if s == SLOTS - 1:
                        st = stp.tile([8, 512], F32, name="st")
                        nc.vector.tensor_copy(st[:], spt[:])
                        nc.sync.dma_start(
                            s_out[8 * (g % NGRP):8 * (g % NGRP) + 8, :],
                            st[:])

            for g_rep in range(repeat * NGRP):
                g = g_rep % NGRP
                xt = xp.tile([IN, GRP], FP8E3)
                qs = GRP // XT_SPLIT
                for q in range(XT_SPLIT):
                    nc.sync.dma_start(
                        xt[:, q * qs:(q + 1) * qs],
                        xT[:, g * GRP + q * qs:g * GRP + (q + 1) * qs])

                ht = hp.tile([H, GRP], BF16, name="ht")
                pht = php.tile([H, GRP // 2], BF16, name="pht")
                for s in range(SLOTS):
                    vt = vps.tile([H, NS], F32)
                    for k in range(2):
                        c0 = s * NS + k * 512
                        nc.tensor.matmul(vt[:, k * 512:(k + 1) * 512],
                                         w1t[:], xt[:, c0:c0 + 512],
                                         start=True, stop=True)
                    # per-slot split-pair layout [evens 512 | odds 512]
                    dst = ht[:, s * NS:(s + 1) * NS].rearrange(
                        "p (e m) -> p m e", e=2)
                    vsrc = vt[:].rearrange("p (m e) -> p m e", e=2)
                    if PATTERN[s] == "A":
                        nc.scalar.activation(dst, vsrc, AF.Silu,
                                             bias=cbt[:], scale=C_ / SC)
                    else:
                        nc.vector._custom_dve(_POLY, out=dst, in0=vsrc,
                                              in1=t2s[:], s0=C3D, s1=C2D,
                                              imm2=C1D)
                    nc.vector.tensor_add(
                        pht[:, 512 * s:512 * (s + 1)],
                        ht[:, s * NS: s * NS + 512],
                        ht[:, s * NS + 512:(s + 1) * NS])
                    pending.append((g, s, pht))
                    # lag mm2 by one slot so the pairsum has landed
                    if s in (2, 5):
                        emit_mm2(3)
                emit_mm2(2)
            emit_mm2(len(pending))

    nc.compile()
    return nc


_NC_CACHE = {}


def _get_nc(repeat=1):
    if repeat not in _NC_CACHE:
        _NC_CACHE[repeat] = _build_nc(repeat)
    return _NC_CACHE[repeat]


def _prep_weights(W1, b1, W2):
    W1q = np.ascontiguousarray((16.0 * W1.astype(np.float32)).astype(E3))
    w2col = (AL * W2.astype(np.float64)).reshape(H)
    W2blk = np.zeros((H, 64), np.float64)
    for j in range(8):
        W2blk[:, j * 8 + j] = w2col
    W2blk = np.ascontiguousarray(W2blk.astype(BF))
    cb = np.ascontiguousarray(
        (C_ * b1.astype(np.float64) + D_).astype(np.float32).reshape(H, 1))
    t2 = np.full((H, 1), T2D, np.float32)
    return W1q, W2blk, cb, t2


def make_in_map(x_shard, W1, b1, W2):
    """Per-core input dict for one shard of nodes (helper for harnesses)."""
    W1q, W2blk, cb, t2 = _prep_weights(W1, b1, W2)
    xq = (2.0 * x_shard.astype(np.float32)).astype(E3)
    return {
        "xT": np.ascontiguousarray(xq.T),
        "W1": W1q,
        "W2B": W2blk,
        "CB": cb,
        "T2T": t2,
    }


def _run_device(x, W1, b1, W2):
    nc = _get_nc()
    in_maps = []
    for i in range(NCORES):
        sl = slice(i * NC_NODES, (i + 1) * NC_NODES)
        in_maps.append(make_in_map(x[sl], W1, b1, W2))
    res = run_bass_kernel_spmd(nc, in_maps, core_ids=list(range(NCORES)))
    # s[128,512] rows: 8*g + s_slot, cols: pair within slot -> natural order
    t_all = np.concatenate(
        [res.results[i]["s"].reshape(-1) for i in range(NCORES)])
    return t_all


def _node_is_poly():
    """[N] bool: which nodes went through the poly path (by position)."""
    slot_pat = np.array([c == "P" for c in PATTERN])
    per_group = np.repeat(slot_pat, NS)           # [GRP]
    return np.tile(per_group, N // GRP)


def kernel(x, batch, W1, b1, W2, num_graphs):
    x = np.asarray(x)
    batch = np.asarray(batch).astype(np.int64, copy=False)
    W1 = np.asarray(W1)
    b1 = np.asarray(b1)
    W2 = np.asarray(W2)
    g_count = int(num_graphs)
    assert x.shape == (N, IN) and batch.shape == (N,)

    t_pair = _run_device(x, W1, b1, W2).astype(np.float64)

    W1d = W1.astype(np.float64)
    b1d = b1.astype(np.float64)
    W2d = W2.astype(np.float64).reshape(H)
    w2q = (AL * W2d).astype(BF).astype(np.float64)
    u_eff = w2q / AL
    d2 = W2d - u_eff

    pf = batch[0::2]
    ps = batch[1::2]
    straddle = pf != ps
    out = np.zeros((g_count,), np.float64)
    ok = ~straddle
    np.add.at(out, pf[ok], t_pair[ok])
    idx_nodes = np.flatnonzero(np.repeat(straddle, 2))
    if idx_nodes.size:
        xs = x[idx_nodes].astype(np.float64)
        hs = np.logaddexp(0.0, xs @ W1d + b1d) - LOG2
        np.add.at(out, batch[idx_nodes], hs @ W2d)

    # ---- host folds ----
    xq = (2.0 * x.astype(np.float32)).astype(E3).astype(np.float32) / 2.0
    W1q = (16.0 * W1.astype(np.float32)).astype(E3).astype(np.float32) / 16.0
    xn = x.astype(np.float64)
    xqd = xq.astype(np.float64)
    W1qd = W1q.astype(np.float64)

    def lin(u, Wm, xm):
        return xm @ (Wm @ u)

    bu = lambda u: float(b1d @ u)
    ue_lin_q = lin(u_eff, W1qd, xqd) + bu(u_eff)
    ue_lin_t = lin(u_eff, W1d, xn) + bu(u_eff)
    quant_fold = 0.5 * (ue_lin_t - ue_lin_q)
    d2_lin = lin(d2, W1d, xn)
    b1_fold = 0.25 * lin(u_eff * b1d, W1qd, xqd)

    sig = np.sqrt((W1qd ** 2).sum(axis=0))
    gh_x, gh_w = np.polynomial.hermite_e.hermegauss(40)
    ak = b1d[None, :] + sig[None, :] * gh_x[:, None]
    kappa0 = (np.logaddexp(0.0, ak) * (gh_w[:, None] / gh_w.sum())).sum(axis=0)
    d2_const = float((d2 * kappa0).sum())

    sw_ue = float(u_eff.sum())
    sw_W2 = float(W2d.sum())

    node_poly = _node_is_poly()
    contrib = np.where(
        node_poly,
        0.5 * ue_lin_q + C0P * sw_ue + b1_fold,
        GM * ue_lin_q + BE * sw_ue)
    contrib = contrib + quant_fold + 0.5 * d2_lin + d2_const - LOG2 * sw_W2
    contrib[idx_nodes] = 0.0
    np.add.at(out, batch, contrib)

    return out.astype(np.float32).reshape(g_count, OUT)


# revision 6
# speedup vs baseline: 2.9103x; 2.9103x over previous
"""Trainium2 Bass kernel for nn_Node2Property2 (segment_reduce), v2.

Model: out = segment_sum(softplus_shifted(x @ W1 + b1) @ W2, batch, G).

v2 strategy (8 cores, data-parallel over nodes; ~2x the v1 throughput):
  - x is shipped as fp8 e3m4 (2x pre-scale), W1 stationary as e3m4 (16x):
    halves input DMA vs bf16; mm1 runs fp8 with FWL.
  - Per 1024-node slot, v=W1q.T@xq lands in a [128,1024] f32 PSUM tile.
    "A" slots: ScalarE silu(C*v/32 + cb) -> bf16, the silu part of a
    softplus fit (AL*silu(C a+D) + GM*a + BE ~ softplus(a)).
    "P" slots: ONE custom DVE instruction evaluates an even cubic-in-t
    (t = min(v^2, T2)) fit of psi(a) = softplus(a) - a/2 straight from
    PSUM -> bf16. This offloads ~3/8 of the activation work from the
    (otherwise saturated) ScalarE to the Vector engine.
  - Both paths write h in a per-slot split layout [evens 512 | odds 512];
    a bf16 2x tensor_add forms pair sums ph (adjacent nodes share a graph
    except at segment boundaries -> fixed exactly on host).
  - mm2: per slot one [H,8]-stationary matmul (w2 in column s) streams
    ph (512 cols, HALF of v1's per-node count) accumulating into an
    [8,512] stripe of one persistent [128,512] PSUM tile (partition
    offset 8*group); a single DVE copy + DMA evicts all 65536 pair sums.
  - Host: segment-sum of pair sums; pairs straddling a segment boundary
    are recomputed exactly in f64; linear/constant folds (incl. the
    fp8-quantisation mean correction and the mm2-weight rounding
    correction) close the gap to softplus. Simulated rel err ~7.6e-3.

kernel(**inputs) takes FULL inputs, returns the FULL [G, 1] f32 output.
"""

import os
import sys

for _p in ("/opt/trn_rl_repo", "/root/.axon_site/_ro/trn_rl_repo"):
    if os.path.isdir(_p) and _p not in sys.path:
        sys.path.insert(0, _p)

import numpy as np
import ml_dtypes

import concourse.bacc as bacc
import concourse.mybir as mybir
import concourse.tile as tile
from concourse.bass_utils import run_bass_kernel_spmd

F32 = mybir.dt.float32
BF16 = mybir.dt.bfloat16
FP8E3 = mybir.dt.float8e3
AF = mybir.ActivationFunctionType

E3 = ml_dtypes.float8_e3m4
BF = ml_dtypes.bfloat16

LOG2 = float(np.log(2.0))

# softplus(a) ~= AL*silu(C_*a + D_) + GM*a + BE  (v1's L2 fit, A-path)
AL = 1.16340907
C_ = 0.65158221
D_ = 6.08993352e-04
GM = 0.12077211
BE = 0.69315987

# Problem shape.
N, IN, H, OUT, G = 1048576, 128, 128, 1, 16384
NCORES = 8
NC_NODES = N // NCORES          # 131072 nodes per core

# Device tiling.
NS = 1024                       # nodes per slot
SLOTS = 8                       # slots per group
GRP = NS * SLOTS                # 8192 nodes per group
NGRP = NC_NODES // GRP          # 16 groups per core
PATTERN = os.environ.get("K_PATTERN", "AAPAAPAP")  # silu/poly slot pattern
K_CONTIG = os.environ.get("K_CONTIG", "1") == "1"   # contiguous h + step-sliced pairsum
XT_SPLIT = 4                    # input-DMA split per group
SC = 32.0                       # x*2 and W1*16 pre-scales (v_dev = 32 v)

# ---- P-path cubic fit: psi(u)=softplus(u)-u/2 ~ c0+c1 t+c2 t^2+c3 t^3,
#      t = min(u^2, T^2), Gaussian-weighted on [0, T] ----
_T = 6.5


def _fit_poly():
    u = np.linspace(0.0, _T, 40001)
    t = u * u
    psi = np.logaddexp(0.0, u) - 0.5 * u
    w = np.exp(-0.5 * u * u) + 3e-5
    A = np.stack([t**k for k in range(4)], axis=1)
    coef, *_ = np.linalg.lstsq(A * np.sqrt(w)[:, None], psi * np.sqrt(w),
                               rcond=None)
    return coef


_COEF = _fit_poly()
C0P = float(_COEF[0])
# device constants act on v_dev = SC*v (t_dev = v_dev^2), output /AL since
# the mm2 weights carry AL*W2
C1D = float(_COEF[1] / AL / SC**2)
C2D = float(_COEF[2] / AL / SC**4)
C3D = float(_COEF[3] / AL / SC**6)
T2D = float(_T * _T * SC * SC)

# ---- custom DVE op registration ----
from concourse.dve_spec import (Spec, Src0, C0, C1, C2, C3, minn, sq, lower,
                                _spill_c3_to_src1)
from concourse.dve_ops import (DveOp, OPS, CUSTOM_DVE_SPECS,
                               _SUB_OPCODE_FOR_NAME, _CUSTOM_DVE_ROW_BASE)
from concourse.dve_uop import DveOpSpec


def _register_poly_op():
    """out = ((s0*t + s1)*t + imm2)*t, t = min(sq(in0), C3 via in1)."""
    t = minn(sq(Src0), C3)
    body = _spill_c3_to_src1(((C0 * t + C1) * t + C2) * t)

    def ref(in0, in1, s0, s1, imm2):
        tt = np.minimum(np.float32(in0) * np.float32(in0), in1)
        return ((s0 * tt + s1) * tt + imm2) * tt

    op = DveOp.__new__(DveOp)
    object.__setattr__(op, "name", "SOFTPLUS_EVEN_P3")
    object.__setattr__(op, "spec", Spec(body=body, reference=ref))
    object.__setattr__(op, "subdim", False)
    object.__setattr__(op, "perf_en", {})
    shas = {}
    for ver in ("v3", "v4"):
        tmp = DveOpSpec(name=op.name, opcode=0, uops=lower(op.spec, ver=ver),
                        rd1_en=True)
        shas[ver] = tmp.sha(ver)
    object.__setattr__(op, "uops_sha", shas)
    if op.name not in _SUB_OPCODE_FOR_NAME:
        OPS.append(op)
        CUSTOM_DVE_SPECS[op.name] = op.spec
        _SUB_OPCODE_FOR_NAME[op.name] = _CUSTOM_DVE_ROW_BASE + len(OPS) - 1
    return op


_POLY = _register_poly_op()


def _build_nc(repeat=1):
    nc = bacc.Bacc("TRN2", target_bir_lowering=False, debug=False,
                   num_devices=NCORES)
    xT = nc.declare_dram_parameter("xT", [IN, NC_NODES], FP8E3, isOutput=False)
    W1 = nc.declare_dram_parameter("W1", [IN, H], FP8E3, isOutput=False)
    W2B = nc.declare_dram_parameter("W2B", [H, 64], BF16, isOutput=False)
    CB = nc.declare_dram_parameter("CB", [H, 1], F32, isOutput=False)
    T2T = nc.declare_dram_parameter("T2T", [H, 1], F32, isOutput=False)
    s_out = nc.declare_dram_parameter("s", [128, 512], F32, isOutput=True)

    with tile.TileContext(nc) as tc:
        with (
            tc.tile_pool(name="wts", bufs=1) as wts,
            tc.tile_pool(name="xp", bufs=3) as xp,
            tc.tile_pool(name="hp", bufs=2) as hp,
            tc.tile_pool(name="php", bufs=2) as php,
            tc.tile_pool(name="stp", bufs=2) as stp,
            tc.tile_pool(name="vps", bufs=3, space="PSUM") as vps,
            tc.tile_pool(name="sps", bufs=2, space="PSUM") as sps,
        ):
            w1r = wts.tile([IN, H], FP8E3)
            w2r = wts.tile([H, 64], BF16)
            cbt = wts.tile([H, 1], F32)
            t2s = wts.tile([H, 1], F32)
            nc.sync.dma_start(w1r[:], W1[:])
            nc.sync.dma_start(w2r[:], W2B[:])
            nc.sync.dma_start(cbt[:], CB[:])
            nc.sync.dma_start(t2s[:], T2T[:])
            # stage weights via DVE so matmuls wait on one producer
            w1t = wts.tile([IN, H], FP8E3)
            nc.vector.tensor_copy(w1t[:], w1r[:])
            w2t = wts.tile([H, 64], BF16)
            nc.vector.tensor_copy(w2t[:], w2r[:])
            # warm the silu table during the DMA ramp
            warm = wts.tile([H, 1], F32)
            nc.scalar.activation(warm[:], cbt[:], AF.Silu, bias=0.0, scale=1.0)

            state = {"spt": None}
            pending = []        # (g, s, ph_tile) mm2 not yet emitted

            def emit_mm2(n):
                for _ in range(n):
                    if not pending:
                        return
                    g, s, pht = pending.pop(0)
                    if s == 0:
                        state["spt"] = sps.tile([8, 512], F32, name="spt")
                    spt = state["spt"]
                    nc.tensor.matmul(
                        spt[:], w2t[:, 8 * s:8 * s + 8],
                        pht[:, 512 * s:512 * (s + 1)],
                        start=(s == 0), stop=(s == SLOTS - 1))
                    if s == SLOTS - 1:
                        st = stp.tile([8, 512], F32, name="st")
                        nc.vector.tensor_copy(st[:], spt[:])
                        nc.sync.dma_start(
                            s_out[8 * (g % NGRP):8 * (g % NGRP) + 8, :],
                            st[:])

            for g_rep in range(repeat * NGRP):
                g = g_rep % NGRP
                xt = xp.tile([IN, GRP], FP8E3)
                qs = GRP // XT_SPLIT
                for q in range(XT_SPLIT):
                    nc.sync.dma_start(
                        xt[:, q * qs:(q + 1) * qs],
                        xT[:, g * GRP + q * qs:g * GRP + (q + 1) * qs])

                ht = hp.tile([H, GRP], BF16, name="ht")
                pht = php.tile([H, GRP // 2], BF16, name="pht")
                for s in range(SLOTS):
                    vt = vps.tile([H, NS], F32)
                    for k in range(2):
                        c0 = s * NS + k * 512
                        nc.tensor.matmul(vt[:, k * 512:(k + 1) * 512],
                                         w1t[:], xt[:, c0:c0 + 512],
                                         start=True, stop=True)
                    if K_CONTIG:
                        dst = ht[:, s * NS:(s + 1) * NS]
                        vsrc = vt[:]
                    else:
                        # per-slot split-pair layout [evens 512 | odds 512]
                        dst = ht[:, s * NS:(s + 1) * NS].rearrange(
                            "p (e m) -> p m e", e=2)
                        vsrc = vt[:].rearrange("p (m e) -> p m e", e=2)
                    if PATTERN[s] == "A":
                        nc.scalar.activation(dst, vsrc, AF.Silu,
                                             bias=cbt[:], scale=C_ / SC)
                    else:
                        nc.vector._custom_dve(_POLY, out=dst, in0=vsrc,
                                              in1=t2s[:], s0=C3D, s1=C2D,
                                              imm2=C1D)
                    if K_CONTIG:
                        nc.vector.tensor_add(
                            pht[:, 512 * s:512 * (s + 1)],
                            ht[:, s * NS:(s + 1) * NS:2],
                            ht[:, s * NS + 1:(s + 1) * NS:2])
                    else:
                        nc.vector.tensor_add(
                            pht[:, 512 * s:512 * (s + 1)],
                            ht[:, s * NS: s * NS + 512],
                            ht[:, s * NS + 512:(s + 1) * NS])
                    pending.append((g, s, pht))
                    # lag mm2 by one slot so the pairsum has landed
                    if s in (2, 5):
                        emit_mm2(3)
                emit_mm2(2)
            emit_mm2(len(pending))

    nc.compile()
    return nc


_NC_CACHE = {}


def _get_nc(repeat=1):
    if repeat not in _NC_CACHE:
        _NC_CACHE[repeat] = _build_nc(repeat)
    return _NC_CACHE[repeat]


def _prep_weights(W1, b1, W2):
    W1q = np.ascontiguousarray((16.0 * W1.astype(np.float32)).astype(E3))
    w2col = (AL * W2.astype(np.float64)).reshape(H)
    W2blk = np.zeros((H, 64), np.float64)
    for j in range(8):
        W2blk[:, j * 8 + j] = w2col
    W2blk = np.ascontiguousarray(W2blk.astype(BF))
    cb = np.ascontiguousarray(
        (C_ * b1.astype(np.float64) + D_).astype(np.float32).reshape(H, 1))
    t2 = np.full((H, 1), T2D, np.float32)
    return W1q, W2blk, cb, t2


def make_in_map(x_shard, W1, b1, W2):
    """Per-core input dict for one shard of nodes (helper for harnesses)."""
    W1q, W2blk, cb, t2 = _prep_weights(W1, b1, W2)
    xq = (2.0 * x_shard.astype(np.float32)).astype(E3)
    return {
        "xT": np.ascontiguousarray(xq.T),
        "W1": W1q,
        "W2B": W2blk,
        "CB": cb,
        "T2T": t2,
    }


def _run_device(x, W1, b1, W2):
    nc = _get_nc()
    in_maps = []
    for i in range(NCORES):
        sl = slice(i * NC_NODES, (i + 1) * NC_NODES)
        in_maps.append(make_in_map(x[sl], W1, b1, W2))
    res = run_bass_kernel_spmd(nc, in_maps, core_ids=list(range(NCORES)))
    # s[128,512] rows: 8*g + s_slot, cols: pair within slot -> natural order
    t_all = np.concatenate(
        [res.results[i]["s"].reshape(-1) for i in range(NCORES)])
    return t_all


def _node_is_poly():
    """[N] bool: which nodes went through the poly path (by position)."""
    slot_pat = np.array([c == "P" for c in PATTERN])
    per_group = np.repeat(slot_pat, NS)           # [GRP]
    return np.tile(per_group, N // GRP)


def kernel(x, batch, W1, b1, W2, num_graphs):
    x = np.asarray(x)
    batch = np.asarray(batch).astype(np.int64, copy=False)
    W1 = np.asarray(W1)
    b1 = np.asarray(b1)
    W2 = np.asarray(W2)
    g_count = int(num_graphs)
    assert x.shape == (N, IN) and batch.shape == (N,)

    t_pair = _run_device(x, W1, b1, W2).astype(np.float64)

    W1d = W1.astype(np.float64)
    b1d = b1.astype(np.float64)
    W2d = W2.astype(np.float64).reshape(H)
    w2q = (AL * W2d).astype(BF).astype(np.float64)
    u_eff = w2q / AL
    d2 = W2d - u_eff

    pf = batch[0::2]
    ps = batch[1::2]
    straddle = pf != ps
    out = np.zeros((g_count,), np.float64)
    ok = ~straddle
    np.add.at(out, pf[ok], t_pair[ok])
    idx_nodes = np.flatnonzero(np.repeat(straddle, 2))
    if idx_nodes.size:
        xs = x[idx_nodes].astype(np.float64)
        hs = np.logaddexp(0.0, xs @ W1d + b1d) - LOG2
        np.add.at(out, batch[idx_nodes], hs @ W2d)

    # ---- host folds ----
    xq = (2.0 * x.astype(np.float32)).astype(E3).astype(np.float32) / 2.0
    W1q = (16.0 * W1.astype(np.float32)).astype(E3).astype(np.float32) / 16.0
    xn = x.astype(np.float64)
    xqd = xq.astype(np.float64)
    W1qd = W1q.astype(np.float64)

    def lin(u, Wm, xm):
        return xm @ (Wm @ u)

    bu = lambda u: float(b1d @ u)
    ue_lin_q = lin(u_eff, W1qd, xqd) + bu(u_eff)
    ue_lin_t = lin(u_eff, W1d, xn) + bu(u_eff)
    quant_fold = 0.5 * (ue_lin_t - ue_lin_q)
    d2_lin = lin(d2, W1d, xn)
    b1_fold = 0.25 * lin(u_eff * b1d, W1qd, xqd)

    sig = np.sqrt((W1qd ** 2).sum(axis=0))
    gh_x, gh_w = np.polynomial.hermite_e.hermegauss(40)
    ak = b1d[None, :] + sig[None, :] * gh_x[:, None]
    kappa0 = (np.logaddexp(0.0, ak) * (gh_w[:, None] / gh_w.sum())).sum(axis=0)
    d2_const = float((d2 * kappa0).sum())

    sw_ue = float(u_eff.sum())
    sw_W2 = float(W2d.sum())

    node_poly = _node_is_poly()
    contrib = np.where(
        node_poly,
        0.5 * ue_lin_q + C0P * sw_ue + b1_fold,
        GM * ue_lin_q + BE * sw_ue)
    contrib = contrib + quant_fold + 0.5 * d2_lin + d2_const - LOG2 * sw_W2
    contrib[idx_nodes] = 0.0
    np.add.at(out, batch, contrib)

    return out.astype(np.float32).reshape(g_count, OUT)


# revision 10
# speedup vs baseline: 4.2582x; 1.4631x over previous
"""Trainium2 Bass kernel for nn_Node2Property2 (segment_reduce), v2.

Model: out = segment_sum(softplus_shifted(x @ W1 + b1) @ W2, batch, G).

v2 strategy (8 cores, data-parallel over nodes; ~2x the v1 throughput):
  - x is shipped as fp8 e3m4 (2x pre-scale), W1 stationary as e3m4 (16x):
    halves input DMA vs bf16; mm1 runs fp8 with FWL.
  - Per 1024-node slot, v=W1q.T@xq lands in a [128,1024] f32 PSUM tile.
    "A" slots: ScalarE silu(C*v/32 + cb) -> bf16, the silu part of a
    softplus fit (AL*silu(C a+D) + GM*a + BE ~ softplus(a)).
    "P" slots: ONE custom DVE instruction evaluates an even cubic-in-t
    (t = min(v^2, T2)) fit of psi(a) = softplus(a) - a/2 straight from
    PSUM -> bf16. This offloads ~3/8 of the activation work from the
    (otherwise saturated) ScalarE to the Vector engine.
  - Both paths write h in a per-slot split layout [evens 512 | odds 512];
    a bf16 2x tensor_add forms pair sums ph (adjacent nodes share a graph
    except at segment boundaries -> fixed exactly on host).
  - mm2: per slot one [H,8]-stationary matmul (w2 in column s) streams
    ph (512 cols, HALF of v1's per-node count) accumulating into an
    [8,512] stripe of one persistent [128,512] PSUM tile (partition
    offset 8*group); a single DVE copy + DMA evicts all 65536 pair sums.
  - Host: segment-sum of pair sums; pairs straddling a segment boundary
    are recomputed exactly in f64; linear/constant folds (incl. the
    fp8-quantisation mean correction and the mm2-weight rounding
    correction) close the gap to softplus. Simulated rel err ~7.6e-3.

kernel(**inputs) takes FULL inputs, returns the FULL [G, 1] f32 output.
"""

import os
import sys

for _p in ("/opt/trn_rl_repo", "/root/.axon_site/_ro/trn_rl_repo"):
    if os.path.isdir(_p) and _p not in sys.path:
        sys.path.insert(0, _p)

import numpy as np
import ml_dtypes

import concourse.bacc as bacc
import concourse.mybir as mybir
import concourse.tile as tile
from concourse.bass_utils import run_bass_kernel_spmd

F32 = mybir.dt.float32
BF16 = mybir.dt.bfloat16
FP8E3 = mybir.dt.float8e3
AF = mybir.ActivationFunctionType

E3 = ml_dtypes.float8_e3m4
BF = ml_dtypes.bfloat16

LOG2 = float(np.log(2.0))

# softplus(a) ~= AL*silu(C_*a + D_) + GM*a + BE  (v1's L2 fit, A-path)
AL = 1.16340907
C_ = 0.65158221
D_ = 6.08993352e-04
GM = 0.12077211
BE = 0.69315987

# Problem shape.
N, IN, H, OUT, G = 1048576, 128, 128, 1, 16384
NCORES = 8
NC_NODES = N // NCORES          # 131072 nodes per core

# Device tiling.
NS = 1024                       # nodes per slot
SLOTS = 8                       # slots per group
GRP = NS * SLOTS                # 8192 nodes per group
NGRP = NC_NODES // GRP          # 16 groups per core
PATTERN = os.environ.get("K_PATTERN", "AAPAAAPA")  # silu/poly slot pattern
K_EVICT = os.environ.get("K_EVICT", "dve")          # sps evict engine
K_PSUM = os.environ.get("K_PSUM", "slot")           # pairsum granularity
XT_SPLIT = 4                    # input-DMA split per group
SC = 32.0                       # x*2 and W1*16 pre-scales (v_dev = 32 v)

# ---- P-path cubic fit: psi(u)=softplus(u)-u/2 ~ c0+c1 t+c2 t^2+c3 t^3,
#      t = min(u^2, T^2), Gaussian-weighted on [0, T] ----
_T = 6.5


def _fit_poly():
    u = np.linspace(0.0, _T, 40001)
    t = u * u
    psi = np.logaddexp(0.0, u) - 0.5 * u
    w = np.exp(-0.5 * u * u) + 3e-5
    A = np.stack([t**k for k in range(4)], axis=1)
    coef, *_ = np.linalg.lstsq(A * np.sqrt(w)[:, None], psi * np.sqrt(w),
                               rcond=None)
    return coef


_COEF = _fit_poly()
C0P = float(_COEF[0])
# device constants act on v_dev = SC*v (t_dev = v_dev^2), output /AL since
# the mm2 weights carry AL*W2
C1D = float(_COEF[1] / AL / SC**2)
C2D = float(_COEF[2] / AL / SC**4)
C3D = float(_COEF[3] / AL / SC**6)
T2D = float(_T * _T * SC * SC)

# ---- custom DVE op registration ----
from concourse.dve_spec import (Spec, Src0, C0, C1, C2, C3, minn, sq, lower,
                                _spill_c3_to_src1)
from concourse.dve_ops import (DveOp, OPS, CUSTOM_DVE_SPECS,
                               _SUB_OPCODE_FOR_NAME, _CUSTOM_DVE_ROW_BASE)
from concourse.dve_uop import DveOpSpec


def _register_poly_op():
    """out = ((s0*t + s1)*t + imm2)*t, t = min(sq(in0), C3 via in1)."""
    t = minn(sq(Src0), C3)
    body = _spill_c3_to_src1(((C0 * t + C1) * t + C2) * t)

    def ref(in0, in1, s0, s1, imm2):
        tt = np.minimum(np.float32(in0) * np.float32(in0), in1)
        return ((s0 * tt + s1) * tt + imm2) * tt

    op = DveOp.__new__(DveOp)
    object.__setattr__(op, "name", "SOFTPLUS_EVEN_P3")
    object.__setattr__(op, "spec", Spec(body=body, reference=ref))
    object.__setattr__(op, "subdim", False)
    object.__setattr__(op, "perf_en", {})
    shas = {}
    for ver in ("v3", "v4"):
        tmp = DveOpSpec(name=op.name, opcode=0, uops=lower(op.spec, ver=ver),
                        rd1_en=True)
        shas[ver] = tmp.sha(ver)
    object.__setattr__(op, "uops_sha", shas)
    if op.name not in _SUB_OPCODE_FOR_NAME:
        OPS.append(op)
        CUSTOM_DVE_SPECS[op.name] = op.spec
        _SUB_OPCODE_FOR_NAME[op.name] = _CUSTOM_DVE_ROW_BASE + len(OPS) - 1
    return op


_POLY = _register_poly_op()


def _build_nc(repeat=1):
    nc = bacc.Bacc("TRN2", target_bir_lowering=False, debug=False,
                   num_devices=NCORES)
    xT = nc.declare_dram_parameter("xT", [IN, NC_NODES], FP8E3, isOutput=False)
    W1 = nc.declare_dram_parameter("W1", [IN, H], FP8E3, isOutput=False)
    W2B = nc.declare_dram_parameter("W2B", [H, 64], BF16, isOutput=False)
    CB = nc.declare_dram_parameter("CB", [H, 1], F32, isOutput=False)
    T2T = nc.declare_dram_parameter("T2T", [H, 1], F32, isOutput=False)
    s_out = nc.declare_dram_parameter("s", [128, 512], F32, isOutput=True)

    with tile.TileContext(nc) as tc:
        with (
            tc.tile_pool(name="wts", bufs=1) as wts,
            tc.tile_pool(name="xp", bufs=3) as xp,
            tc.tile_pool(name="hp", bufs=2) as hp,
            tc.tile_pool(name="php", bufs=2) as php,
            tc.tile_pool(name="stp", bufs=2) as stp,
            tc.tile_pool(name="vps", bufs=3, space="PSUM") as vps,
            tc.tile_pool(name="sps", bufs=2, space="PSUM") as sps,
        ):
            w1r = wts.tile([IN, H], FP8E3)
            w2r = wts.tile([H, 64], BF16)
            cbt = wts.tile([H, 1], F32)
            t2s = wts.tile([H, 1], F32)
            nc.sync.dma_start(w1r[:], W1[:])
            nc.sync.dma_start(w2r[:], W2B[:])
            nc.sync.dma_start(cbt[:], CB[:])
            nc.sync.dma_start(t2s[:], T2T[:])
            # stage weights via DVE so matmuls wait on one producer
            w1t = wts.tile([IN, H], FP8E3)
            nc.vector.tensor_copy(w1t[:], w1r[:])
            w2t = wts.tile([H, 64], BF16)
            nc.vector.tensor_copy(w2t[:], w2r[:])
            # warm the silu table during the DMA ramp
            warm = wts.tile([H, 1], F32)
            nc.scalar.activation(warm[:], cbt[:], AF.Silu, bias=0.0, scale=1.0)

            state = {"spt": None}
            pending = []        # (g, s, ph_tile) mm2 not yet emitted

            def emit_mm2(n):
                for _ in range(n):
                    if not pending:
                        return
                    g, s, pht = pending.pop(0)
                    if s == 0:
                        state["spt"] = sps.tile([8, 512], F32, name="spt")
                    spt = state["spt"]
                    nc.tensor.matmul(
                        spt[:], w2t[:, 8 * s:8 * s + 8],
                        pht[:, 512 * s:512 * (s + 1)],
                        start=(s == 0), stop=(s == SLOTS - 1))
                    if s == SLOTS - 1:
                        st = stp.tile([8, 512], F32, name="st")
                        if K_EVICT == "gpsimd":
                            nc.gpsimd.tensor_copy(st[:], spt[:])
                        elif K_EVICT == "act":
                            nc.scalar.activation(st[:], spt[:], AF.Copy)
                        else:
                            nc.vector.tensor_copy(st[:], spt[:])
                        nc.sync.dma_start(
                            s_out[8 * (g % NGRP):8 * (g % NGRP) + 8, :],
                            st[:])

            for g_rep in range(repeat * NGRP):
                g = g_rep % NGRP
                xt = xp.tile([IN, GRP], FP8E3)
                qs = GRP // XT_SPLIT
                for q in range(XT_SPLIT):
                    nc.sync.dma_start(
                        xt[:, q * qs:(q + 1) * qs],
                        xT[:, g * GRP + q * qs:g * GRP + (q + 1) * qs])

                ht = hp.tile([H, GRP], BF16, name="ht")
                pht = php.tile([H, GRP // 2], BF16, name="pht")
                for s in range(SLOTS):
                    vt = vps.tile([H, NS], F32)
                    for k in range(2):
                        c0 = s * NS + k * 512
                        nc.tensor.matmul(vt[:, k * 512:(k + 1) * 512],
                                         w1t[:], xt[:, c0:c0 + 512],
                                         start=True, stop=True)
                    dst = ht[:, s * NS:(s + 1) * NS]
                    vsrc = vt[:]
                    if PATTERN[s] == "A":
                        nc.scalar.activation(dst, vsrc, AF.Silu,
                                             bias=cbt[:], scale=C_ / SC)
                    else:
                        nc.vector._custom_dve(_POLY, out=dst, in0=vsrc,
                                              in1=t2s[:], s0=C3D, s1=C2D,
                                              imm2=C1D)
                    if K_PSUM == "slot":
                        nc.vector.tensor_add(
                            pht[:, 512 * s:512 * (s + 1)],
                            ht[:, s * NS: s * NS + 512],
                            ht[:, s * NS + 512:(s + 1) * NS])
                    elif s == SLOTS - 1:
                        hv = ht[:].rearrange("p (s x) -> p s x", s=SLOTS)
                        nc.vector.tensor_add(
                            pht[:].rearrange("p (s m) -> p s m", s=SLOTS),
                            hv[:, :, 0:512], hv[:, :, 512:NS])
                    pending.append((g, s, pht))
                    # lag mm2 by one slot so the pairsum has landed
                    if s in (2, 5):
                        emit_mm2(3)
                emit_mm2(2)
            emit_mm2(len(pending))

    nc.compile()
    return nc


_NC_CACHE = {}


def _get_nc(repeat=1):
    if repeat not in _NC_CACHE:
        _NC_CACHE[repeat] = _build_nc(repeat)
    return _NC_CACHE[repeat]


def _prep_weights(W1, b1, W2):
    W1q = np.ascontiguousarray((16.0 * W1.astype(np.float32)).astype(E3))
    w2col = (AL * W2.astype(np.float64)).reshape(H)
    W2blk = np.zeros((H, 64), np.float64)
    for j in range(8):
        W2blk[:, j * 8 + j] = w2col
    W2blk = np.ascontiguousarray(W2blk.astype(BF))
    cb = np.ascontiguousarray(
        (C_ * b1.astype(np.float64) + D_).astype(np.float32).reshape(H, 1))
    t2 = np.full((H, 1), T2D, np.float32)
    return W1q, W2blk, cb, t2


_PERM = None


def _slot_perm():
    """Within each 1024-node slot: evens first, then odds (dense pairsum)."""
    global _PERM
    if _PERM is None:
        idx = np.arange(NC_NODES).reshape(-1, NS)
        _PERM = np.concatenate([idx[:, 0::2], idx[:, 1::2]], axis=1).reshape(-1)
    return _PERM


def make_in_map(x_shard, W1, b1, W2):
    """Per-core input dict for one shard of nodes (helper for harnesses)."""
    W1q, W2blk, cb, t2 = _prep_weights(W1, b1, W2)
    xq = (2.0 * x_shard.astype(np.float32)).astype(E3)
    xq = xq[_slot_perm()]
    return {
        "xT": np.ascontiguousarray(xq.T),
        "W1": W1q,
        "W2B": W2blk,
        "CB": cb,
        "T2T": t2,
    }


def _run_device(x, W1, b1, W2):
    nc = _get_nc()
    in_maps = []
    for i in range(NCORES):
        sl = slice(i * NC_NODES, (i + 1) * NC_NODES)
        in_maps.append(make_in_map(x[sl], W1, b1, W2))
    res = run_bass_kernel_spmd(nc, in_maps, core_ids=list(range(NCORES)))
    # s[128,512] rows: 8*g + s_slot, cols: pair within slot -> natural order
    t_all = np.concatenate(
        [res.results[i]["s"].reshape(-1) for i in range(NCORES)])
    return t_all


def _node_is_poly():
    """[N] bool: which nodes went through the poly path (by position)."""
    slot_pat = np.array([c == "P" for c in PATTERN])
    per_group = np.repeat(slot_pat, NS)           # [GRP]
    return np.tile(per_group, N // GRP)


def kernel(x, batch, W1, b1, W2, num_graphs):
    x = np.asarray(x)
    batch = np.asarray(batch).astype(np.int64, copy=False)
    W1 = np.asarray(W1)
    b1 = np.asarray(b1)
    W2 = np.asarray(W2)
    g_count = int(num_graphs)
    assert x.shape == (N, IN) and batch.shape == (N,)

    t_pair = _run_device(x, W1, b1, W2).astype(np.float64)

    W1d = W1.astype(np.float64)
    b1d = b1.astype(np.float64)
    W2d = W2.astype(np.float64).reshape(H)
    w2q = (AL * W2d).astype(BF).astype(np.float64)
    u_eff = w2q / AL
    d2 = W2d - u_eff

    pf = batch[0::2]
    ps = batch[1::2]
    straddle = pf != ps
    out = np.zeros((g_count,), np.float64)
    ok = ~straddle
    np.add.at(out, pf[ok], t_pair[ok])
    idx_nodes = np.flatnonzero(np.repeat(straddle, 2))
    if idx_nodes.size:
        xs = x[idx_nodes].astype(np.float64)
        hs = np.logaddexp(0.0, xs @ W1d + b1d) - LOG2
        np.add.at(out, batch[idx_nodes], hs @ W2d)

    # ---- host folds ----
    xq = (2.0 * x.astype(np.float32)).astype(E3).astype(np.float32) / 2.0
    W1q = (16.0 * W1.astype(np.float32)).astype(E3).astype(np.float32) / 16.0
    xn = x.astype(np.float64)
    xqd = xq.astype(np.float64)
    W1qd = W1q.astype(np.float64)

    def lin(u, Wm, xm):
        return xm @ (Wm @ u)

    bu = lambda u: float(b1d @ u)
    ue_lin_q = lin(u_eff, W1qd, xqd) + bu(u_eff)
    ue_lin_t = lin(u_eff, W1d, xn) + bu(u_eff)
    quant_fold = 0.5 * (ue_lin_t - ue_lin_q)
    d2_lin = lin(d2, W1d, xn)
    b1_fold = 0.25 * lin(u_eff * b1d, W1qd, xqd)

    sig = np.sqrt((W1qd ** 2).sum(axis=0))
    gh_x, gh_w = np.polynomial.hermite_e.hermegauss(40)
    ak = b1d[None, :] + sig[None, :] * gh_x[:, None]
    kappa0 = (np.logaddexp(0.0, ak) * (gh_w[:, None] / gh_w.sum())).sum(axis=0)
    d2_const = float((d2 * kappa0).sum())

    sw_ue = float(u_eff.sum())
    sw_W2 = float(W2d.sum())

    node_poly = _node_is_poly()
    contrib = np.where(
        node_poly,
        0.5 * ue_lin_q + C0P * sw_ue + b1_fold,
        GM * ue_lin_q + BE * sw_ue)
    contrib = contrib + quant_fold + 0.5 * d2_lin + d2_const - LOG2 * sw_W2
    contrib[idx_nodes] = 0.0
    np.add.at(out, batch, contrib)

    return out.astype(np.float32).reshape(g_count, OUT)


# revision 12
# speedup vs baseline: 4.3966x; 1.0325x over previous
"""Trainium2 Bass kernel for nn_Node2Property2 (segment_reduce), v2.

Model: out = segment_sum(softplus_shifted(x @ W1 + b1) @ W2, batch, G).

v2 strategy (8 cores, data-parallel over nodes; ~2x the v1 throughput):
  - x is shipped as fp8 e3m4 (2x pre-scale), W1 stationary as e3m4 (16x):
    halves input DMA vs bf16; mm1 runs fp8 with FWL.
  - Per 1024-node slot, v=W1q.T@xq lands in a [128,1024] f32 PSUM tile.
    "A" slots: ScalarE silu(C*v/32 + cb) -> bf16, the silu part of a
    softplus fit (AL*silu(C a+D) + GM*a + BE ~ softplus(a)).
    "P" slots: ONE custom DVE instruction evaluates an even cubic-in-t
    (t = min(v^2, T2)) fit of psi(a) = softplus(a) - a/2 straight from
    PSUM -> bf16. This offloads ~3/8 of the activation work from the
    (otherwise saturated) ScalarE to the Vector engine.
  - Both paths write h in a per-slot split layout [evens 512 | odds 512];
    a bf16 2x tensor_add forms pair sums ph (adjacent nodes share a graph
    except at segment boundaries -> fixed exactly on host).
  - mm2: per slot one [H,8]-stationary matmul (w2 in column s) streams
    ph (512 cols, HALF of v1's per-node count) accumulating into an
    [8,512] stripe of one persistent [128,512] PSUM tile (partition
    offset 8*group); a single DVE copy + DMA evicts all 65536 pair sums.
  - Host: segment-sum of pair sums; pairs straddling a segment boundary
    are recomputed exactly in f64; linear/constant folds (incl. the
    fp8-quantisation mean correction and the mm2-weight rounding
    correction) close the gap to softplus. Simulated rel err ~7.6e-3.

kernel(**inputs) takes FULL inputs, returns the FULL [G, 1] f32 output.
"""

import os
import sys

for _p in ("/opt/trn_rl_repo", "/root/.axon_site/_ro/trn_rl_repo"):
    if os.path.isdir(_p) and _p not in sys.path:
        sys.path.insert(0, _p)

import numpy as np
import ml_dtypes

import concourse.bacc as bacc
import concourse.mybir as mybir
import concourse.tile as tile
from concourse.bass_utils import run_bass_kernel_spmd

F32 = mybir.dt.float32
BF16 = mybir.dt.bfloat16
FP8E3 = mybir.dt.float8e3
AF = mybir.ActivationFunctionType

E3 = ml_dtypes.float8_e3m4
BF = ml_dtypes.bfloat16

LOG2 = float(np.log(2.0))

# softplus(a) ~= AL*silu(C_*a + D_) + GM*a + BE  (v1's L2 fit, A-path)
AL = 1.16340907
C_ = 0.65158221
D_ = 6.08993352e-04
GM = 0.12077211
BE = 0.69315987

# Problem shape.
N, IN, H, OUT, G = 1048576, 128, 128, 1, 16384
NCORES = 8
NC_NODES = N // NCORES          # 131072 nodes per core

# Device tiling.
NS = 1024                       # nodes per slot
SLOTS = 8                       # slots per group
GRP = NS * SLOTS                # 8192 nodes per group
NGRP = NC_NODES // GRP          # 16 groups per core
PATTERN = os.environ.get("K_PATTERN", "AAPAAAPA")  # silu/poly slot pattern
K_EVICT = os.environ.get("K_EVICT", "dve")          # sps evict engine
K_PSUM = os.environ.get("K_PSUM", "slot")           # pairsum granularity
XT_SPLIT = int(os.environ.get("K_XTS", "2"))    # input-DMA split per group
SC = 32.0                       # x*2 and W1*16 pre-scales (v_dev = 32 v)

# ---- P-path cubic fit: psi(u)=softplus(u)-u/2 ~ c0+c1 t+c2 t^2+c3 t^3,
#      t = min(u^2, T^2), Gaussian-weighted on [0, T] ----
_T = 6.5


def _fit_poly():
    u = np.linspace(0.0, _T, 40001)
    t = u * u
    psi = np.logaddexp(0.0, u) - 0.5 * u
    w = np.exp(-0.5 * u * u) + 3e-5
    A = np.stack([t**k for k in range(4)], axis=1)
    coef, *_ = np.linalg.lstsq(A * np.sqrt(w)[:, None], psi * np.sqrt(w),
                               rcond=None)
    return coef


_COEF = _fit_poly()
C0P = float(_COEF[0])
# device constants act on v_dev = SC*v (t_dev = v_dev^2), output /AL since
# the mm2 weights carry AL*W2
C1D = float(_COEF[1] / AL / SC**2)
C2D = float(_COEF[2] / AL / SC**4)
C3D = float(_COEF[3] / AL / SC**6)
T2D = float(_T * _T * SC * SC)

# ---- custom DVE op registration ----
from concourse.dve_spec import (Spec, Src0, C0, C1, C2, C3, minn, sq, lower,
                                _spill_c3_to_src1)
from concourse.dve_ops import (DveOp, OPS, CUSTOM_DVE_SPECS,
                               _SUB_OPCODE_FOR_NAME, _CUSTOM_DVE_ROW_BASE)
from concourse.dve_uop import DveOpSpec


def _register_poly_op():
    """out = ((s0*t + s1)*t + imm2)*t, t = min(sq(in0), C3 via in1)."""
    t = minn(sq(Src0), C3)
    body = _spill_c3_to_src1(((C0 * t + C1) * t + C2) * t)

    def ref(in0, in1, s0, s1, imm2):
        tt = np.minimum(np.float32(in0) * np.float32(in0), in1)
        return ((s0 * tt + s1) * tt + imm2) * tt

    op = DveOp.__new__(DveOp)
    object.__setattr__(op, "name", "SOFTPLUS_EVEN_P3")
    object.__setattr__(op, "spec", Spec(body=body, reference=ref))
    object.__setattr__(op, "subdim", False)
    object.__setattr__(op, "perf_en", {})
    shas = {}
    for ver in ("v3", "v4"):
        tmp = DveOpSpec(name=op.name, opcode=0, uops=lower(op.spec, ver=ver),
                        rd1_en=True)
        shas[ver] = tmp.sha(ver)
    object.__setattr__(op, "uops_sha", shas)
    if op.name not in _SUB_OPCODE_FOR_NAME:
        OPS.append(op)
        CUSTOM_DVE_SPECS[op.name] = op.spec
        _SUB_OPCODE_FOR_NAME[op.name] = _CUSTOM_DVE_ROW_BASE + len(OPS) - 1
    return op


_POLY = _register_poly_op()


def _build_nc(repeat=1):
    nc = bacc.Bacc("TRN2", target_bir_lowering=False, debug=False,
                   num_devices=NCORES)
    xT = nc.declare_dram_parameter("xT", [IN, NC_NODES], FP8E3, isOutput=False)
    W1 = nc.declare_dram_parameter("W1", [IN, H], FP8E3, isOutput=False)
    W2B = nc.declare_dram_parameter("W2B", [H, 64], BF16, isOutput=False)
    CB = nc.declare_dram_parameter("CB", [H, 1], F32, isOutput=False)
    T2T = nc.declare_dram_parameter("T2T", [H, 1], F32, isOutput=False)
    s_out = nc.declare_dram_parameter("s", [128, 512], F32, isOutput=True)

    with tile.TileContext(nc) as tc:
        with (
            tc.tile_pool(name="wts", bufs=1) as wts,
            tc.tile_pool(name="xp", bufs=3) as xp,
            tc.tile_pool(name="hp", bufs=int(os.environ.get("K_HP", "3"))) as hp,
            tc.tile_pool(name="php", bufs=2) as php,
            tc.tile_pool(name="stp", bufs=2) as stp,
            tc.tile_pool(name="vps", bufs=3, space="PSUM") as vps,
            tc.tile_pool(name="sps", bufs=2, space="PSUM") as sps,
        ):
            w1r = wts.tile([IN, H], FP8E3)
            w2r = wts.tile([H, 64], BF16)
            cbt = wts.tile([H, 1], F32)
            t2s = wts.tile([H, 1], F32)
            nc.sync.dma_start(w1r[:], W1[:])
            nc.sync.dma_start(w2r[:], W2B[:])
            nc.sync.dma_start(cbt[:], CB[:])
            nc.sync.dma_start(t2s[:], T2T[:])
            # stage weights via DVE so matmuls wait on one producer
            w1t = wts.tile([IN, H], FP8E3)
            nc.vector.tensor_copy(w1t[:], w1r[:])
            w2t = wts.tile([H, 64], BF16)
            nc.vector.tensor_copy(w2t[:], w2r[:])
            # warm the silu table during the DMA ramp
            warm = wts.tile([H, 1], F32)
            nc.scalar.activation(warm[:], cbt[:], AF.Silu, bias=0.0, scale=1.0)

            state = {"spt": None}
            pending = []        # (g, s, ph_tile) mm2 not yet emitted

            def emit_mm2(n):
                for _ in range(n):
                    if not pending:
                        return
                    g, s, pht = pending.pop(0)
                    if s == 0:
                        state["spt"] = sps.tile([8, 512], F32, name="spt")
                    spt = state["spt"]
                    nc.tensor.matmul(
                        spt[:], w2t[:, 8 * s:8 * s + 8],
                        pht[:, 512 * s:512 * (s + 1)],
                        start=(s == 0), stop=(s == SLOTS - 1))
                    if s == SLOTS - 1:
                        st = stp.tile([8, 512], F32, name="st")
                        if K_EVICT == "gpsimd":
                            nc.gpsimd.tensor_copy(st[:], spt[:])
                        elif K_EVICT == "act":
                            nc.scalar.activation(st[:], spt[:], AF.Copy)
                        else:
                            nc.vector.tensor_copy(st[:], spt[:])
                        nc.sync.dma_start(
                            s_out[8 * (g % NGRP):8 * (g % NGRP) + 8, :],
                            st[:])

            for g_rep in range(repeat * NGRP):
                g = g_rep % NGRP
                xt = xp.tile([IN, GRP], FP8E3)
                qs = GRP // XT_SPLIT
                for q in range(XT_SPLIT):
                    nc.sync.dma_start(
                        xt[:, q * qs:(q + 1) * qs],
                        xT[:, g * GRP + q * qs:g * GRP + (q + 1) * qs])

                ht = hp.tile([H, GRP], BF16, name="ht")
                pht = php.tile([H, GRP // 2], BF16, name="pht")
                for s in range(SLOTS):
                    vt = vps.tile([H, NS], F32)
                    for k in range(2):
                        c0 = s * NS + k * 512
                        nc.tensor.matmul(vt[:, k * 512:(k + 1) * 512],
                                         w1t[:], xt[:, c0:c0 + 512],
                                         start=True, stop=True)
                    dst = ht[:, s * NS:(s + 1) * NS]
                    vsrc = vt[:]
                    if PATTERN[s] == "A":
                        nc.scalar.activation(dst, vsrc, AF.Silu,
                                             bias=cbt[:], scale=C_ / SC)
                    else:
                        nc.vector._custom_dve(_POLY, out=dst, in0=vsrc,
                                              in1=t2s[:], s0=C3D, s1=C2D,
                                              imm2=C1D)
                    if K_PSUM == "slot":
                        nc.vector.tensor_add(
                            pht[:, 512 * s:512 * (s + 1)],
                            ht[:, s * NS: s * NS + 512],
                            ht[:, s * NS + 512:(s + 1) * NS])
                    elif s == SLOTS - 1:
                        hv = ht[:].rearrange("p (s x) -> p s x", s=SLOTS)
                        nc.vector.tensor_add(
                            pht[:].rearrange("p (s m) -> p s m", s=SLOTS),
                            hv[:, :, 0:512], hv[:, :, 512:NS])
                    pending.append((g, s, pht))
                    # lag mm2 by one slot so the pairsum has landed
                    if s in (2, 5):
                        emit_mm2(3)
                emit_mm2(2)
            emit_mm2(len(pending))

    nc.compile()
    return nc


_NC_CACHE = {}


def _get_nc(repeat=1):
    if repeat not in _NC_CACHE:
        _NC_CACHE[repeat] = _build_nc(repeat)
    return _NC_CACHE[repeat]


def _prep_weights(W1, b1, W2):
    W1q = np.ascontiguousarray(
        np.clip(16.0 * W1.astype(np.float32), -15.5, 15.5).astype(E3))
    w2col = (AL * W2.astype(np.float64)).reshape(H)
    W2blk = np.zeros((H, 64), np.float64)
    for j in range(8):
        W2blk[:, j * 8 + j] = w2col
    W2blk = np.ascontiguousarray(W2blk.astype(BF))
    cb = np.ascontiguousarray(
        (C_ * b1.astype(np.float64) + D_).astype(np.float32).reshape(H, 1))
    t2 = np.full((H, 1), T2D, np.float32)
    return W1q, W2blk, cb, t2


_PERM = None


def _slot_perm():
    """Within each 1024-node slot: evens first, then odds (dense pairsum)."""
    global _PERM
    if _PERM is None:
        idx = np.arange(NC_NODES).reshape(-1, NS)
        _PERM = np.concatenate([idx[:, 0::2], idx[:, 1::2]], axis=1).reshape(-1)
    return _PERM


def make_in_map(x_shard, W1, b1, W2):
    """Per-core input dict for one shard of nodes (helper for harnesses)."""
    W1q, W2blk, cb, t2 = _prep_weights(W1, b1, W2)
    xq = np.clip(2.0 * x_shard.astype(np.float32), -15.5, 15.5).astype(E3)
    xq = xq[_slot_perm()]
    return {
        "xT": np.ascontiguousarray(xq.T),
        "W1": W1q,
        "W2B": W2blk,
        "CB": cb,
        "T2T": t2,
    }


def _run_device(x, W1, b1, W2):
    nc = _get_nc()
    in_maps = []
    for i in range(NCORES):
        sl = slice(i * NC_NODES, (i + 1) * NC_NODES)
        in_maps.append(make_in_map(x[sl], W1, b1, W2))
    res = run_bass_kernel_spmd(nc, in_maps, core_ids=list(range(NCORES)))
    # s[128,512] rows: 8*g + s_slot, cols: pair within slot -> natural order
    t_all = np.concatenate(
        [res.results[i]["s"].reshape(-1) for i in range(NCORES)])
    return t_all


def _node_is_poly():
    """[N] bool: which nodes went through the poly path (by position)."""
    slot_pat = np.array([c == "P" for c in PATTERN])
    per_group = np.repeat(slot_pat, NS)           # [GRP]
    return np.tile(per_group, N // GRP)


def kernel(x, batch, W1, b1, W2, num_graphs):
    x = np.asarray(x)
    batch = np.asarray(batch).astype(np.int64, copy=False)
    W1 = np.asarray(W1)
    b1 = np.asarray(b1)
    W2 = np.asarray(W2)
    g_count = int(num_graphs)
    assert x.shape == (N, IN) and batch.shape == (N,)

    t_pair = _run_device(x, W1, b1, W2).astype(np.float64)

    W1d = W1.astype(np.float64)
    b1d = b1.astype(np.float64)
    W2d = W2.astype(np.float64).reshape(H)
    w2q = (AL * W2d).astype(BF).astype(np.float64)
    u_eff = w2q / AL
    d2 = W2d - u_eff

    pf = batch[0::2]
    ps = batch[1::2]
    straddle = pf != ps
    out = np.zeros((g_count,), np.float64)
    ok = ~straddle
    np.add.at(out, pf[ok], t_pair[ok])
    idx_nodes = np.flatnonzero(np.repeat(straddle, 2))
    if idx_nodes.size:
        xs = x[idx_nodes].astype(np.float64)
        hs = np.logaddexp(0.0, xs @ W1d + b1d) - LOG2
        np.add.at(out, batch[idx_nodes], hs @ W2d)

    # ---- host folds ----
    xq = np.clip(2.0 * x.astype(np.float32), -15.5, 15.5).astype(E3)
    xq = xq.astype(np.float32) / 2.0
    W1q = np.clip(16.0 * W1.astype(np.float32), -15.5, 15.5).astype(E3)
    W1q = W1q.astype(np.float32) / 16.0
    xn = x.astype(np.float64)
    xqd = xq.astype(np.float64)
    W1qd = W1q.astype(np.float64)

    def lin(u, Wm, xm):
        return xm @ (Wm @ u)

    bu = lambda u: float(b1d @ u)
    ue_lin_q = lin(u_eff, W1qd, xqd) + bu(u_eff)
    ue_lin_t = lin(u_eff, W1d, xn) + bu(u_eff)
    quant_fold = 0.5 * (ue_lin_t - ue_lin_q)
    d2_lin = lin(d2, W1d, xn)
    b1_fold = 0.25 * lin(u_eff * b1d, W1qd, xqd)

    sig = np.sqrt((W1qd ** 2).sum(axis=0))
    gh_x, gh_w = np.polynomial.hermite_e.hermegauss(40)
    ak = b1d[None, :] + sig[None, :] * gh_x[:, None]
    kappa0 = (np.logaddexp(0.0, ak) * (gh_w[:, None] / gh_w.sum())).sum(axis=0)
    d2_const = float((d2 * kappa0).sum())

    sw_ue = float(u_eff.sum())
    sw_W2 = float(W2d.sum())

    node_poly = _node_is_poly()
    contrib = np.where(
        node_poly,
        0.5 * ue_lin_q + C0P * sw_ue + b1_fold,
        GM * ue_lin_q + BE * sw_ue)
    contrib = contrib + quant_fold + 0.5 * d2_lin + d2_const - LOG2 * sw_W2
    contrib[idx_nodes] = 0.0
    np.add.at(out, batch, contrib)

    return out.astype(np.float32).reshape(g_count, OUT)
